# revision 10
# baseline (speedup 1.0000x reference)
"""GroupedQueryAttention Trainium2 kernel (8 NeuronCores).

Sharding: (batch b in 0..1) x (kv-head group g in 0..3) -> core 4*b+g.
Each core computes, for its batch, the 4 query heads (4g..4g+3) that share
kv head g, plus the partial output projection through the matching 512-row
slice of Wo.  The host sums the 4 partials per batch.

On-device dataflow is fully "transposed": activations live as [feature,
token] so every matmul contraction sits on the partition axis, and the
softmax probabilities come out directly in the layout the P@V matmul
needs.  Softmax denominators come from an all-ones stationary matmul over
the probability tiles (pre-broadcast across partitions).  Causality is
exploited by only computing score tiles on/below the block diagonal.

v2 optimizations over the baseline:
- fp8e4m3 DoubleRow matmuls for the q/k projections (the 32x weight
  prescale cancels inside rmsnorm) and for the off-block-diagonal
  probability @ V / denominator matmuls (2 contraction tiles per pass).
- exp computed with a -2 bias so fp8 probabilities can't overflow; the
  shift cancels between numerator and denominator per chunk.
- softmax reciprocal via the ~5x faster approx-fast DVE op; rmsnorm via
  Sqrt + approx reciprocal (keeps the Act engine on one function table).
- per-chunk double-buffered x tiles and pair-granular first-chunk DMA so
  the Tensor engine starts ~1us after launch.
- block-diagonal mask multiplies paired into [P,1024] ops.
- bf16 partial output (halves the output DMA).
"""

import numpy as np
import ml_dtypes

DIM, H, KV, S, B = 2048, 16, 4, 2048, 2
HD = DIM // H          # 128
GQ = H // KV           # 4 query heads per kv head
P = 128                # partitions
NK = DIM // P          # 16 contraction tiles
NCH = S // 512         # 4 sequence chunks of 512
EPS = 1e-6
BF = ml_dtypes.bfloat16
F8 = ml_dtypes.float8_e4m3fn
W8SCALE = 32.0
EXP_BIAS = -2.0

FP8Q = True    # q projection in fp8 DoubleRow
FP8K = True    # k projection in fp8 DoubleRow
FP8PV = True   # off-diagonal P@V + denominator in fp8 DoubleRow

_CACHED = {}


def _build_program(fp8q=FP8Q, fp8k=FP8K, fp8pv=FP8PV):
    import concourse.bass as bass
    import concourse.tile as tile
    from concourse import bacc
    from concourse import mybir
    from concourse.masks import make_identity

    f32 = mybir.dt.float32
    bf16 = mybir.dt.bfloat16
    f8 = mybir.dt.float8e4
    AF = mybir.ActivationFunctionType
    DR = mybir.MatmulPerfMode.DoubleRow

    any8 = fp8q or fp8k

    nc = bacc.Bacc()
    xT = nc.declare_dram_parameter("xT", [DIM, S], bf16, isOutput=False)
    if any8:
        xT8 = nc.declare_dram_parameter("xT8", [DIM, S], f8, isOutput=False)
    # bf16 weights always present: chunk 0 (few-key queries, no softmax
    # averaging to suppress quantization noise) projects in bf16.
    wq = nc.declare_dram_parameter("wq", [DIM, GQ * HD], bf16, isOutput=False)
    wk = nc.declare_dram_parameter("wk", [DIM, HD], bf16, isOutput=False)
    if fp8q:
        wq8 = nc.declare_dram_parameter("wq8", [DIM, GQ * HD], f8, isOutput=False)
    if fp8k:
        wk8 = nc.declare_dram_parameter("wk8", [DIM, HD], f8, isOutput=False)
    wv = nc.declare_dram_parameter("wv", [DIM, HD], bf16, isOutput=False)
    wo = nc.declare_dram_parameter("wo", [GQ * HD, DIM], bf16, isOutput=False)
    cosq = nc.declare_dram_parameter("cosq", [HD, S], bf16, isOutput=False)
    sinq = nc.declare_dram_parameter("sinq", [HD, S], bf16, isOutput=False)
    cosk = nc.declare_dram_parameter("cosk", [HD, S], bf16, isOutput=False)
    sink = nc.declare_dram_parameter("sink", [HD, S], bf16, isOutput=False)
    m4 = nc.declare_dram_parameter("m4", [4, P, 512], bf16, isOutput=False)
    rsw = nc.declare_dram_parameter("rsw", [P, P], bf16, isOutput=False)
    po = nc.declare_dram_parameter("po", [S, DIM], bf16, isOutput=True)

    inv_sqrt_hd = 1.0 / float(np.sqrt(HD))

    with tile.TileContext(nc) as tc:
      with tc.tile_pool(name="const", bufs=1) as const, \
           tc.tile_pool(name="hatp", bufs=1) as hatp, \
           tc.tile_pool(name="w5", bufs=2) as w5, \
           tc.tile_pool(name="m4p", bufs=1) as m4p, \
           tc.tile_pool(name="csp", bufs=1) as csp:
        ones_sb = const.tile([P, P], bf16)
        nc.vector.memset(ones_sb, 1.0)
        ident = const.tile([P, P], bf16)
        make_identity(nc, ident)
        rsw_sb = const.tile([P, P], bf16)
        nc.scalar.dma_start(out=rsw_sb, in_=rsw[:, :])
        epsb = const.tile([P, 1], f32)
        nc.vector.memset(epsb, EPS)
        ebias = const.tile([P, 1], f32)
        nc.vector.memset(ebias, EXP_BIAS)
        if fp8pv:
            ones8 = const.tile([P, 2, P], f8)
            nc.vector.memset(ones8, 1.0)

        # prefetched during P1 (emitted after chunk 0 so they queue behind it)
        wo_sb = w5.tile([P, GQ, DIM], bf16, bufs=1)
        m4_sb = m4p.tile([P, 4, 512], bf16)
        cs_sb = {}
        for nm in ("cosq", "sinq", "cosk", "sink"):
            cs_sb[nm] = csp.tile([P, S], bf16, tag=f"cs_{nm}", name=f"cs_{nm}")

        v_nat = hatp.tile([P, NK, HD], bf16, tag="vnat")
        if fp8pv:
            v8_nat = hatp.tile([P, NK, HD], f8, tag="v8nat")
        onorm = [hatp.tile([P, S], bf16, tag=f"onorm{h}", name=f"onorm{h}")
                 for h in range(GQ)]
        qhat = [hatp.tile([P, S], bf16, tag=f"qhat{h}", name=f"qhat{h}")
                for h in range(GQ)]
        khat = hatp.tile([P, S], bf16, tag="khat")

        with tc.tile_pool(name="qkvp", bufs=1) as qkvp:
            q32 = [qkvp.tile([P, S], bf16, tag=f"qp_{h}", name=f"qp_{h}")
                   for h in range(GQ)]
            k32 = qkvp.tile([P, S], bf16, tag="kp")
            vT_bf = qkvp.tile([P, S], bf16, tag="vT")

            # ---- P1: projections (transposed outputs) + v transpose ----
            with tc.tile_pool(name="wtp", bufs=1) as wtp, \
                 tc.tile_pool(name="xcp", bufs=2) as xcp, \
                 tc.tile_pool(name="p1ps", bufs=3, space="PSUM") as p1ps:
                wk_sb = wtp.tile([P, NK, HD], bf16)
                wq_sb = wtp.tile([P, NK, GQ * HD], bf16)
                wv_sb = wtp.tile([P, NK, HD], bf16)
                wk8_sb = wtp.tile([P, NK, HD], f8, name="wk8_sb") if fp8k else None
                wq8_sb = wtp.tile([P, NK, GQ * HD], f8, name="wq8_sb") if fp8q else None
                wk_src = wk.ap().rearrange("(j p) n -> p j n", p=P)
                xt_src = xT.ap().rearrange("(j p) t -> p j t", p=P)
                if any8:
                    xt8_src = xT8.ap().rearrange("(j p) t -> p j t", p=P)

                # chunk 0 inputs at j-pair granularity so the first matmul
                # can start ~1us in; later chunks as whole-chunk DMAs that
                # prefetch behind compute (xcp bufs=2).  chunk 0 projects in
                # bf16 only, so it needs no fp8 x tile.
                xc8 = [None] * NCH
                xcb = [None] * NCH

                def load_chunk(c):
                    csl = slice(c * 512, (c + 1) * 512)
                    xcb[c] = xcp.tile([P, NK, 512], bf16, tag="xb",
                                      name=f"xb_{c}")
                    if c == 0:
                        for jj in range(NK // 2):
                            js = slice(2 * jj, 2 * jj + 2)
                            nc.sync.dma_start(out=xcb[c][:, js, :],
                                              in_=xt_src[:, js, csl])
                        return
                    if any8:
                        xc8[c] = xcp.tile([P, NK, 512], f8, tag="x8",
                                          name=f"x8_{c}")
                        nc.sync.dma_start(out=xc8[c], in_=xt8_src[:, :, csl])
                    nc.sync.dma_start(out=xcb[c], in_=xt_src[:, :, csl])

                # weights: wk first (pair-granular), wq, wv, then fp8 copies
                for jj in range(NK // 2):
                    js = slice(2 * jj, 2 * jj + 2)
                    nc.scalar.dma_start(out=wk_sb[:, js, :], in_=wk_src[:, js, :])
                load_chunk(0)
                nc.scalar.dma_start(out=wq_sb,
                                    in_=wq.ap().rearrange("(j p) n -> p j n", p=P))
                nc.scalar.dma_start(out=wv_sb,
                                    in_=wv.ap().rearrange("(j p) n -> p j n", p=P))
                if fp8k:
                    nc.scalar.dma_start(
                        out=wk8_sb, in_=wk8.ap().rearrange("(j p) n -> p j n", p=P))
                if fp8q:
                    nc.scalar.dma_start(
                        out=wq8_sb, in_=wq8.ap().rearrange("(j p) n -> p j n", p=P))
                load_chunk(1)

                for c in range(NCH):
                    sl = slice(c * 512, (c + 1) * 512)
                    if c + 2 < NCH:
                        load_chunk(c + 2)
                    # k first: khat is needed by every score tile in P3
                    for slot in (4, 0, 1, 2, 3, 5):
                        if slot < 4:
                            cols, is8 = slice(slot * HD, (slot + 1) * HD), fp8q and c > 0
                            w_sb = wq8_sb if is8 else wq_sb
                        elif slot == 4:
                            cols, is8 = slice(0, HD), fp8k and c > 0
                            w_sb = wk8_sb if is8 else wk_sb
                        else:
                            cols, is8 = slice(0, HD), False
                            w_sb = wv_sb
                        ps = p1ps.tile([P, 512], f32, tag="proj")
                        if is8:
                            for jj in range(NK // 2):
                                js = slice(2 * jj, 2 * jj + 2)
                                nc.tensor.matmul(ps, w_sb[:, js, cols],
                                                 xc8[c][:, js, :],
                                                 start=(jj == 0),
                                                 stop=(jj == NK // 2 - 1),
                                                 perf_mode=DR)
                        else:
                            for j in range(NK):
                                nc.tensor.matmul(ps, w_sb[:, j, cols],
                                                 xcb[c][:, j, :],
                                                 start=(j == 0), stop=(j == NK - 1))
                        if slot < 4:
                            nc.scalar.copy(q32[slot][:, sl], ps)
                        elif slot == 4:
                            nc.scalar.copy(k32[:, sl], ps)
                        else:
                            nc.scalar.copy(vT_bf[:, sl], ps)
                    if c == 0:
                        nc.scalar.dma_start(
                            out=wo_sb, in_=wo.ap().rearrange("(h p) n -> p h n", p=P))
                        nc.scalar.dma_start(
                            out=m4_sb, in_=m4.ap().rearrange("a p n -> p a n"))
                        for nm, t in (("cosq", cosq), ("sinq", sinq),
                                      ("cosk", cosk), ("sink", sink)):
                            nc.scalar.dma_start(out=cs_sb[nm], in_=t[:, :])
                # v natural layout [sk_local, j, d] via PE transpose of vT
                for j in range(NK):
                    tp = p1ps.tile([P, HD], bf16, tag="vtr")
                    nc.tensor.transpose(tp, vT_bf[:, j * HD:(j + 1) * HD], ident)
                    nc.scalar.copy(v_nat[:, j, :], tp)
                if fp8pv:
                    nc.vector.tensor_copy(
                        v8_nat.rearrange("p j n -> p (j n)"),
                        v_nat.rearrange("p j n -> p (j n)"))

            # ---- P2: rmsnorm (pre-gain) + rope, full-row ops, k first ----
            with tc.tile_pool(name="w2", bufs=2) as w2, \
                 tc.tile_pool(name="p2ps", bufs=2, space="PSUM") as p2ps:
                for t in (4, 0, 1, 2, 3):
                    src = q32[t] if t < 4 else k32
                    dst = qhat[t] if t < 4 else khat
                    cosT = cs_sb["cosq" if t < 4 else "cosk"]
                    sinT = cs_sb["sinq" if t < 4 else "sink"]
                    # sum of squares over feature (partition) axis via
                    # all-ones matmul; arrives replicated on all partitions
                    sqb = w2.tile([P, S], bf16, tag="sqb")
                    nc.scalar.activation(sqb, src, AF.Square)
                    ssq = p2ps.tile([P, S], f32, tag="ssq", bufs=1)
                    rot = p2ps.tile([P, S // 2], f32, tag="rot", bufs=2)
                    rot2 = p2ps.tile([P, S // 2], f32, tag="rot", bufs=2)
                    for c in range(NCH):
                        sl = slice(c * 512, (c + 1) * 512)
                        nc.tensor.matmul(ssq[:, sl], ones_sb, sqb[:, sl],
                                         start=True, stop=True)
                        rt = rot if c < 2 else rot2
                        rsl = slice((c % 2) * 512, (c % 2 + 1) * 512)
                        nc.tensor.matmul(rt[:, rsl], rsw_sb, src[:, sl],
                                         start=True, stop=True)
                    # rms = sqrt(ssq/HD + eps) on Act (any table set has
                    # sqrt's set incl. square); 1/rms via approx DVE recip.
                    # (the fp8 weight prescale cancels here: rsb ~ 1/32x)
                    srms = w2.tile([P, S], f32, tag="srms")
                    nc.scalar.activation(srms, ssq, AF.Sqrt, bias=epsb,
                                         scale=1.0 / HD)
                    rsb = w2.tile([P, S], f32, tag="rsb")
                    nc.vector.reciprocal_approx_fast(out=rsb, in_=srms)
                    # rope: y = src*cos + rot(src)*sin (sign/gain in tables)
                    t1 = w2.tile([P, S], bf16, tag="t1")
                    nc.vector.tensor_mul(t1, src, cosT)
                    t2 = w2.tile([P, S], bf16, tag="t2")
                    nc.vector.tensor_mul(t2[:, 0:1024], rot, sinT[:, 0:1024])
                    nc.vector.tensor_mul(t2[:, 1024:2048], rot2, sinT[:, 1024:2048])
                    t3 = w2.tile([P, S], bf16, tag="t3")
                    nc.vector.tensor_add(t3, t1, t2)
                    nc.vector.tensor_mul(dst, t3, rsb)

        # ---- P3: attention, all heads per chunk ----
        with tc.tile_pool(name="wep", bufs=2) as wep:
          with tc.tile_pool(name="ptp", bufs=18) as ptp, \
               tc.tile_pool(name="pt8p", bufs=26) as pt8p, \
               tc.tile_pool(name="p3s", bufs=2, space="PSUM") as p3s, \
               tc.tile_pool(name="p3o", bufs=4, space="PSUM") as p3o:
              for c in range(NCH):
                  sl = slice(c * 512, (c + 1) * 512)
                  nj = 4 * c + 4
                  npr = nj // 2
                  # scores + exp; two 512-wide sk-tiles per PSUM tile so the
                  # exp runs 1024 wide.  Off-diagonal tiles quantize to fp8
                  # (no mask needed); block-diagonal tiles stay bf16 and get
                  # the paired 0/1 mask multiply.  exp has a -2 bias so fp8
                  # can't overflow; the shift cancels via the denominator.
                  ptsc = {}
                  for h in range(GQ):
                      for pr in range(npr):
                          diag = pr >= 2 * c
                          sc = p3s.tile([P, 1024], f32, tag="sc",
                                        name=f"sc_{c}_{h}_{pr}")
                          for u in range(2):
                              j = 2 * pr + u
                              nc.tensor.matmul(sc[:, u * 512:(u + 1) * 512],
                                               khat[:, j * P:(j + 1) * P],
                                               qhat[h][:, sl],
                                               start=True, stop=True)
                          if diag or not fp8pv:
                              pt = ptp.tile([P, 1024], bf16, tag="pt",
                                            name=f"pt_{c}_{h}_{pr}")
                          else:
                              pt = pt8p.tile([P, 1024], f8, tag="pt8",
                                             name=f"pt8_{c}_{h}_{pr}")
                          nc.scalar.activation(pt, sc, AF.Exp,
                                               bias=ebias, scale=inv_sqrt_hd)
                          if diag:
                              a = pr - 2 * c  # 0 or 1 -> mask pair
                              nc.vector.tensor_mul(
                                  pt, pt,
                                  m4_sb[:, 2 * a:2 * a + 2, :].rearrange(
                                      "p a n -> p (a n)"))
                          ptsc[(h, pr)] = pt
                  # P@V, pr-outer so the stationary v tile is reused across
                  # heads; off-diagonal pairs via fp8 DoubleRow.
                  ots = [p3o.tile([P, 512], f32, tag="ot", name=f"ot_{c}_{h}")
                         for h in range(GQ)]
                  for pr in range(npr):
                      diag = pr >= 2 * c
                      if fp8pv and not diag:
                          for h in range(GQ):
                              nc.tensor.matmul(
                                  ots[h], v8_nat[:, 2 * pr:2 * pr + 2, :],
                                  ptsc[(h, pr)].rearrange("p (a n) -> p a n", a=2),
                                  start=(pr == 0), stop=False, perf_mode=DR)
                      else:
                          for u in range(2):
                              j = 2 * pr + u
                              usl = slice(u * 512, (u + 1) * 512)
                              for h in range(GQ):
                                  nc.tensor.matmul(
                                      ots[h], v_nat[:, j, :],
                                      ptsc[(h, pr)][:, usl],
                                      start=(pr == 0 and u == 0),
                                      stop=(pr == npr - 1 and u == 1))
                  # denominators (replicated across partitions by the
                  # all-ones stationary; reuse sc slots), then normalize
                  for h in range(GQ):
                      den = p3s.tile([P, 512], f32, tag="sc", name=f"den_{c}_{h}")
                      for pr in range(npr):
                          diag = pr >= 2 * c
                          if fp8pv and not diag:
                              nc.tensor.matmul(
                                  den, ones8,
                                  ptsc[(h, pr)].rearrange("p (a n) -> p a n", a=2),
                                  start=(pr == 0), stop=False, perf_mode=DR)
                          else:
                              for u in range(2):
                                  usl = slice(u * 512, (u + 1) * 512)
                                  nc.tensor.matmul(
                                      den, ones_sb, ptsc[(h, pr)][:, usl],
                                      start=(pr == 0 and u == 0),
                                      stop=(pr == npr - 1 and u == 1))
                      rec = wep.tile([P, 512], f32, tag="rec")
                      nc.vector.reciprocal_approx_fast(out=rec, in_=den)
                      nc.vector.tensor_mul(onorm[h][:, sl], ots[h], rec)

          # ---- P5: partial output projection: po = onorm^T @ Wo_g ----
          with tc.tile_pool(name="p5ps", bufs=8, space="PSUM") as p5ps:
              for i in range(S // P):
                  isl = slice(i * P, (i + 1) * P)
                  po_ps = [p5ps.tile([P, 512], f32, tag="po", name=f"po_{i}_{n2}")
                           for n2 in range(NCH)]
                  for h in range(GQ):
                      for n in range(NCH):
                          nc.tensor.matmul(po_ps[n], onorm[h][:, isl],
                                           wo_sb[:, h, n * 512:(n + 1) * 512],
                                           start=(h == 0), stop=(h == GQ - 1))
                  row = wep.tile([P, DIM], bf16, tag="row")
                  for n in range(NCH):
                      if n % 2 == 0:
                          nc.scalar.copy(row[:, n * 512:(n + 1) * 512], po_ps[n])
                      else:
                          nc.vector.tensor_copy(row[:, n * 512:(n + 1) * 512], po_ps[n])
                  nc.sync.dma_start(out=po[isl, :], in_=row)
    nc.compile()
    return nc


def _causal_ok(mask):
    m = np.asarray(mask).reshape(S, S)
    tri = np.tril(np.ones((S, S), dtype=bool))
    return bool(np.all(m[tri] == 0.0) and np.all(m[~tri] <= -1e8))


def _reference_fallback(x, Wq, Wk, Wv, Wo, qg, kg, cos, sin, mask):
    x64 = np.asarray(x, dtype=np.float32)
    q = (x64 @ Wq).reshape(B, S, H, HD).transpose(0, 2, 1, 3)
    k = (x64 @ Wk).reshape(B, S, KV, HD).transpose(0, 2, 1, 3)
    v = (x64 @ Wv).reshape(B, S, KV, HD).transpose(0, 2, 1, 3)

    def rms(t, g):
        r = np.sqrt(np.mean(t * t, axis=-1, keepdims=True) + EPS)
        return g * (t / r)

    q, k = rms(q, qg), rms(k, kg)

    def rot(t):
        return np.concatenate([-t[..., HD // 2:], t[..., :HD // 2]], axis=-1)

    c = cos[None, None, :, :]
    s = sin[None, None, :, :]
    q = q * c + rot(q) * s
    k = k * c + rot(k) * s
    k = np.repeat(k, GQ, axis=1)
    v = np.repeat(v, GQ, axis=1)
    sc = np.einsum('bhqd,bhkd->bhqk', q, k) / np.sqrt(HD) + np.asarray(mask).reshape(1, 1, S, S)
    sc = sc - sc.max(axis=-1, keepdims=True)
    e = np.exp(sc)
    a = e / e.sum(axis=-1, keepdims=True)
    o = np.einsum('bhqk,bhkd->bhqd', a, v)
    o = o.transpose(0, 2, 1, 3).reshape(B, S, H * HD)
    return (o @ Wo).astype(np.float32)


def _make_inmaps(x, Wq, Wk, Wv, Wo, qg, kg, cos, sin):
    cosT = np.ascontiguousarray(cos.T)  # [HD, S]
    sinT = np.ascontiguousarray(sin.T)

    # rope via halves: out[:64] = x[:64]*cos[:64] + x[64:]*sin_tbl[:64]
    #                  out[64:] = x[64:]*cos[64:] + x[:64]*sin_tbl[64:]
    # reference: rot(x)[:64] = -x[64:], rot(x)[64:] = x[:64]; gains fold in.
    def tables(g):
        ct = cosT * g[:, None]
        st = np.empty_like(sinT)
        st[:64] = -sinT[:64] * g[64:, None]
        st[64:] = sinT[64:] * g[:64, None]
        return ct.astype(BF), st.astype(BF)

    cq, sq = tables(qg)
    ck, sk = tables(kg)

    rswm = np.zeros((P, P), dtype=np.float32)
    for i in range(P):
        rswm[i, (i + 64) % P] = 1.0
    rswm = rswm.astype(BF)

    cols = np.arange(512)[None, :]
    rows = np.arange(P)[:, None]
    m4 = np.stack([(cols - P * a >= rows) for a in range(4)]).astype(BF)

    xT = [np.ascontiguousarray(x[b].T).astype(BF) for b in range(B)]
    xT8 = [np.ascontiguousarray(x[b].T).astype(F8) for b in range(B)]

    in_maps = []
    for core in range(8):
        b, g = divmod(core, KV)
        wq_s = np.ascontiguousarray(Wq[:, g * GQ * HD:(g + 1) * GQ * HD])
        wk_s = np.ascontiguousarray(Wk[:, g * HD:(g + 1) * HD])
        m = {
            "xT": xT[b],
            "wq": wq_s.astype(BF),
            "wk": wk_s.astype(BF),
            "wv": np.ascontiguousarray(Wv[:, g * HD:(g + 1) * HD]).astype(BF),
            "wo": np.ascontiguousarray(Wo[g * GQ * HD:(g + 1) * GQ * HD, :]).astype(BF),
            "cosq": cq, "sinq": sq, "cosk": ck, "sink": sk,
            "m4": m4, "rsw": rswm,
        }
        if FP8Q:
            m["wq8"] = (wq_s * W8SCALE).astype(F8)
        if FP8K:
            m["wk8"] = (wk_s * W8SCALE).astype(F8)
        if FP8Q or FP8K:
            m["xT8"] = xT8[b]
        in_maps.append(m)
    return in_maps


def _check_rows(out, x, Wv, Wo):
    """Cheap corruption guard: for query 0 the causal softmax is exactly
    [1.0], so out[b,0] = repeat(x[b,0] @ Wv) @ Wo.  Catches the transient
    whole-run corruption occasionally seen on a freshly booted device."""
    for b in range(B):
        v0 = x[b, 0].astype(np.float32) @ Wv.astype(np.float32)   # [512]
        o_full = np.repeat(v0.reshape(KV, HD), GQ, axis=0).reshape(H * HD)
        exp_row = o_full @ Wo.astype(np.float32)
        got = out[b, 0]
        err = np.abs(got - exp_row).max() / (np.abs(exp_row).max() + 1e-9)
        if err > 0.05:
            return False
    return True


def kernel(x, Wq, Wk, Wv, Wo, qg, kg, cos, sin, mask, **_unused):
    x = np.asarray(x, dtype=np.float32)
    Wq, Wk, Wv, Wo = (np.asarray(a, dtype=np.float32) for a in (Wq, Wk, Wv, Wo))
    qg, kg = np.asarray(qg, np.float32), np.asarray(kg, np.float32)
    cos, sin = np.asarray(cos, np.float32), np.asarray(sin, np.float32)
    if not _causal_ok(mask):
        return _reference_fallback(x, Wq, Wk, Wv, Wo, qg, kg, cos, sin, mask)

    from concourse.bass_utils import run_bass_kernel_spmd

    if "nc" not in _CACHED:
        _CACHED["nc"] = _build_program()
    nc = _CACHED["nc"]

    in_maps = _make_inmaps(x, Wq, Wk, Wv, Wo, qg, kg, cos, sin)

    for attempt in range(3):
        res = run_bass_kernel_spmd(nc, in_maps, list(range(8)))
        out = np.zeros((B, S, DIM), dtype=np.float32)
        for core in range(8):
            out[core // KV] += np.asarray(res.results[core]["po"],
                                          dtype=np.float32)
        if _check_rows(out, x, Wv, Wo):
            break
    return out
